# revision 1
# baseline (speedup 1.0000x reference)
"""Causal self-attention with RoPE for TRN2, sharded over 8 NeuronCores.

Sharding (Megatron-style tensor parallel on heads):
  - 16 heads -> 2 heads per core; each core also handles both batch rows.
  - Each core computes q/k/v projections for its 2 heads (256 features),
    causal attention for its (b, h) pairs, and a partial output
    projection through its 256 columns of Wo.
  - Host sums the 8 partial outputs (the "all-reduce").

All matmuls run as float32r (full-rate fp32 on the PE array). Everything
is kept in transposed layouts so no attention-side transposes are needed:
  qT/kT/vT: [hd=128, S]   scoresT: [j, q]   attnT: [j, q]   outT: [d, q]
Softmax runs without max-subtraction (scaled scores are O(6), exp is safe);
partition-axis sums use a ones-column matmul; 1/sum is broadcast back to
128 partitions with a K=1 ones-row matmul and folded into the outT evict.
"""
import sys

sys.path.insert(0, "/opt/trn_rl_repo")

import numpy as np
import ml_dtypes

import concourse.bass as bass
import concourse.bacc as bacc
import concourse.mybir as mybir
import concourse.tile as tile
from concourse.bass_utils import run_bass_kernel_spmd

F32 = mybir.dt.float32
F32R = mybir.dt.float32r

B, S, D, H, HD = 2, 2048, 2048, 16, 128
N_CORES = 8
HPC = H // N_CORES          # heads per core = 2
FPC = HPC * HD              # features per core = 256
SCALE = 1.0 / float(np.sqrt(HD))
NKT = D // 128              # 16 contraction tiles
NSC = S // 512              # 4 s-chunks per batch
NJT = S // 128              # 16 key tiles per batch
BS = B * S


def _round_f32r(x):
    x = np.ascontiguousarray(x, dtype=np.float32)
    hi = x.astype(ml_dtypes.bfloat16).astype(np.float32)
    lo = (x - hi).astype(ml_dtypes.bfloat16).astype(np.float32)
    return hi + lo


def build_nc():
    nc = bacc.Bacc(None, target_bir_lowering=False, debug=False)
    Exp = mybir.ActivationFunctionType.Exp

    xT_d = nc.dram_tensor("xT", [D, BS], F32R, kind="ExternalInput")
    wq_d = nc.dram_tensor("wq", [D, FPC], F32R, kind="ExternalInput")
    wk_d = nc.dram_tensor("wk", [D, FPC], F32R, kind="ExternalInput")
    wv_d = nc.dram_tensor("wv", [D, FPC], F32R, kind="ExternalInput")
    wo_d = nc.dram_tensor("wo", [FPC, D], F32R, kind="ExternalInput")
    cos_d = nc.dram_tensor("cos", [128, S], F32R, kind="ExternalInput")
    sin_d = nc.dram_tensor("sin", [128, S], F32R, kind="ExternalInput")
    mask_d = nc.dram_tensor("mask", [128, 896], F32R, kind="ExternalInput")
    rmat_d = nc.dram_tensor("rmat", [128, 128], F32R, kind="ExternalInput")
    ident_d = nc.dram_tensor("ident", [128, 128], F32R, kind="ExternalInput")
    onesc_d = nc.dram_tensor("onesc", [128, 1], F32R, kind="ExternalInput")
    onesr_d = nc.dram_tensor("onesr", [1, 128], F32R, kind="ExternalInput")
    out_d = nc.dram_tensor("outP", [D, BS], F32, kind="ExternalOutput")

    # group kt tiles in fours so each x DMA moves 1 MiB in one descriptor
    xT_r = xT_d[:].rearrange("(n t p) s -> n p t s", p=128, t=4)
    wq_r = wq_d[:].rearrange("(g t p) f -> g p t f", p=128, t=4)
    wk_r = wk_d[:].rearrange("(g t p) f -> g p t f", p=128, t=4)
    wv_r = wv_d[:].rearrange("(g t p) f -> g p t f", p=128, t=4)
    wo_r = wo_d[:].rearrange("(ft p) d -> p ft d", p=128)
    out_r = out_d[:].rearrange("(dt p) s -> dt p s", p=128)

    with tile.TileContext(nc) as tc:
        with (
            nc.allow_low_precision(reason="f32r matmul rounding is intended"),
            tc.tile_pool(name="const", bufs=1) as constp,
            tc.tile_pool(name="xt", bufs=2) as xtp,
            tc.tile_pool(name="qkv", bufs=1) as qkvp,
            tc.tile_pool(name="vh", bufs=2) as vhp,
            tc.tile_pool(name="rope", bufs=2) as ropep,
            tc.tile_pool(name="attn", bufs=5) as attnp,
            tc.tile_pool(name="small", bufs=2) as smallp,
            tc.tile_pool(name="osb", bufs=1) as osbp,
            tc.tile_pool(name="wot", bufs=2) as wotp,
            tc.tile_pool(name="outev", bufs=3) as outevp,
            tc.tile_pool(name="pacc", bufs=6, space="PSUM") as paccp,
            tc.tile_pool(name="pav", bufs=1, space="PSUM") as pavp,
            tc.tile_pool(name="psum1", bufs=1, space="PSUM") as psum1p,
        ):
            # ---- constants ----
            wq_g, wk_g, wv_g = [], [], []
            for g in range(NKT // 4):
                wqt = constp.tile([128, 4, FPC], F32R, name=f"wq_g{g}")
                wkt = constp.tile([128, 4, FPC], F32R, name=f"wk_g{g}")
                wvt = constp.tile([128, 4, FPC], F32R, name=f"wv_g{g}")
                weng = nc.sync if g == 0 else nc.scalar
                weng.dma_start(wqt[:], wq_r[g])
                weng.dma_start(wkt[:], wk_r[g])
                weng.dma_start(wvt[:], wv_r[g])
                wq_g.append(wqt); wk_g.append(wkt); wv_g.append(wvt)
            cos_sb = constp.tile([128, S], F32R)
            sin_sb = constp.tile([128, S], F32R)
            nc.scalar.dma_start(cos_sb[:], cos_d[:])
            nc.scalar.dma_start(sin_sb[:], sin_d[:])
            mask_sb = constp.tile([128, 896], F32R)
            nc.scalar.dma_start(mask_sb[:], mask_d[:])
            rmat_sb = constp.tile([128, 128], F32R)
            ident_sb = constp.tile([128, 128], F32R)
            onesc_sb = constp.tile([128, 1], F32R)
            onesr_sb = constp.tile([1, 128], F32R)
            nc.scalar.dma_start(rmat_sb[:], rmat_d[:])
            nc.scalar.dma_start(ident_sb[:], ident_d[:])
            nc.scalar.dma_start(onesc_sb[:], onesc_d[:])
            nc.scalar.dma_start(onesr_sb[:], onesr_d[:])

            for b in range(B):
                o_sb = osbp.tile([128, HPC, S], F32R, tag="o_sb")
                qTs, kTs, vTs = [], [], []
                for h in range(HPC):
                    qTs.append(qkvp.tile([128, S], F32R, name=f"qT{h}", tag=f"qT{h}"))
                    kTs.append(qkvp.tile([128, S], F32R, name=f"kT{h}", tag=f"kT{h}"))
                    vTs.append(qkvp.tile([128, S], F32R, name=f"vT{h}", tag=f"vT{h}"))
                # ---- projections: both heads share each xT tile ----
                for sc in range(NSC):
                    ss = slice(512 * sc, 512 * sc + 512)
                    acc = [paccp.tile([128, 512], F32, name=f"pa{j}", tag="pacc")
                           for j in range(6)]
                    for g in range(NKT // 4):
                        xt = xtp.tile([128, 4, 512], F32R, tag="xt")
                        eng = nc.sync if g % 2 == 0 else nc.gpsimd
                        eng.dma_start(
                            xt[:], xT_r[g, :, :, b * S + 512 * sc:
                                        b * S + 512 * sc + 512])
                        for i in range(4):
                            kt = 4 * g + i
                            st, sp = kt == 0, kt == NKT - 1
                            for h in range(HPC):
                                fs = slice(128 * h, 128 * h + 128)
                                nc.tensor.matmul(acc[h][:], wq_g[g][:, i, fs],
                                                 xt[:, i, :], start=st, stop=sp)
                                nc.tensor.matmul(acc[2 + h][:], wk_g[g][:, i, fs],
                                                 xt[:, i, :], start=st, stop=sp)
                                nc.tensor.matmul(acc[4 + h][:], wv_g[g][:, i, fs],
                                                 xt[:, i, :], start=st, stop=sp)
                    for h in range(HPC):
                        nc.scalar.copy(qTs[h][:, ss], acc[h][:])
                        nc.scalar.copy(kTs[h][:, ss], acc[2 + h][:])
                        nc.scalar.copy(vTs[h][:, ss], acc[4 + h][:])
                for h in range(HPC):
                    qT, kT, vT = qTs[h], kTs[h], vTs[h]
                    # ---- RoPE in place on qT, kT ----
                    for t_ in (qT, kT):
                        for sc in range(NSC):
                            ss = slice(512 * sc, 512 * sc + 512)
                            ps_rot = paccp.tile([128, 512], F32, tag="pacc")
                            nc.tensor.matmul(ps_rot[:], rmat_sb[:], t_[:, ss],
                                             start=True, stop=True)
                            t2 = ropep.tile([128, 512], F32R, tag="ropetmp")
                            nc.vector.tensor_mul(t2[:], ps_rot[:], sin_sb[:, ss])
                            t1 = ropep.tile([128, 512], F32R, tag="ropetmp")
                            nc.vector.tensor_mul(t1[:], t_[:, ss], cos_sb[:, ss])
                            nc.vector.tensor_add(t_[:, ss], t1[:], t2[:])
                    # ---- transpose vT -> v_h [j, jt, d] ----
                    v_h = vhp.tile([128, NJT, 128], F32R, tag="v_h")
                    for jt in range(NJT):
                        js = slice(128 * jt, 128 * jt + 128)
                        ps_tp = paccp.tile([128, 128], F32R, tag="pacc")
                        nc.tensor.transpose(ps_tp[:], vT[:, js], ident_sb[:])
                        nc.scalar.copy(v_h[:, jt, :], ps_tp[:])
                    # ---- attention ----
                    for qc in range(NSC):
                        qs = slice(512 * qc, 512 * qc + 512)
                        ps_av = pavp.tile([128, 512], F32, tag="pav")
                        ps_sum = psum1p.tile([1, 512], F32, tag="psum1")
                        njt = 4 * qc + 4
                        for jt in range(njt):
                            js = slice(128 * jt, 128 * jt + 128)
                            ps_sc = paccp.tile([128, 512], F32, tag="pacc")
                            nc.tensor.matmul(ps_sc[:], kT[:, js], qT[:, qs],
                                             start=True, stop=True)
                            at = attnp.tile([128, 512], F32R, tag="at")
                            nc.scalar.activation(at[:], ps_sc[:], Exp,
                                                 scale=SCALE)
                            if jt >= 4 * qc:
                                mi = 384 - 128 * (jt - 4 * qc)
                                nc.vector.tensor_mul(
                                    at[:], at[:], mask_sb[:, mi:mi + 512])
                            st, sp = jt == 0, jt == njt - 1
                            nc.tensor.matmul(ps_sum[:], onesc_sb[:], at[:],
                                             start=st, stop=sp)
                            nc.tensor.matmul(ps_av[:], v_h[:, jt, :], at[:],
                                             start=st, stop=sp)
                        sums_sb = smallp.tile([1, 512], F32, tag="sums")
                        nc.scalar.copy(sums_sb[:], ps_sum[:])
                        recip = smallp.tile([1, 512], F32R, tag="recip")
                        nc.vector.reciprocal(recip[:], sums_sb[:])
                        ps_bc = paccp.tile([128, 512], F32, tag="pacc")
                        nc.tensor.matmul(ps_bc[:], onesr_sb[:], recip[:],
                                         start=True, stop=True)
                        recipT = smallp.tile([128, 512], F32, tag="recipT")
                        nc.scalar.copy(recipT[:], ps_bc[:])
                        nc.vector.tensor_mul(o_sb[:, h, qs], ps_av[:],
                                             recipT[:])
                # ---- output projection partial for batch b ----
                for dt in range(D // 128):
                    ds = slice(128 * dt, 128 * dt + 128)
                    wo01 = wotp.tile([128, 2, 128], F32R, tag="wo_t")
                    nc.gpsimd.dma_start(wo01[:], wo_r[:, :, ds])
                    for half in range(2):
                        outt = outevp.tile([128, 1024], F32, tag="outt")
                        for j in range(2):
                            sc = 2 * half + j
                            ss = slice(512 * sc, 512 * sc + 512)
                            ps_o = paccp.tile([128, 512], F32, tag="pacc")
                            nc.tensor.matmul(ps_o[:], wo01[:, 0, :],
                                             o_sb[:, 0, ss],
                                             start=True, stop=False)
                            nc.tensor.matmul(ps_o[:], wo01[:, 1, :],
                                             o_sb[:, 1, ss],
                                             start=False, stop=True)
                            nc.vector.tensor_copy(outt[:, 512 * j:512 * j + 512],
                                                  ps_o[:])
                        oeng = nc.sync if (dt + half) % 2 == 0 else nc.gpsimd
                        oeng.dma_start(
                            out_r[dt, :, b * S + 1024 * half:
                                  b * S + 1024 * half + 1024], outt[:])

    nc.compile()
    return nc


_NC_CACHE = None


def _get_nc():
    global _NC_CACHE
    if _NC_CACHE is None:
        _NC_CACHE = build_nc()
    return _NC_CACHE


def _host_consts():
    inv_freq = 1.0 / (10000.0 ** (np.arange(0, HD, 2, dtype=np.float32) / HD))
    t = np.arange(S, dtype=np.float32)
    freqs = np.outer(t, inv_freq)
    emb = np.concatenate([freqs, freqs], axis=-1)          # [S, hd]
    cosT = _round_f32r(np.cos(emb).T)                       # [hd, S]
    sinT = _round_f32r(np.sin(emb).T)
    # staircase mask: variant i is the slice [:, 384-128i : 384-128i+512]
    r = np.arange(128)[:, None]
    u = np.arange(896)[None, :]
    mask = (u >= r + 384).astype(np.float32)
    rmat = np.zeros((128, 128), np.float32)
    for m in range(64):
        rmat[m + 64, m] = -1.0
        rmat[m, m + 64] = 1.0
    ident = np.eye(128, dtype=np.float32)
    onesc = np.ones((128, 1), np.float32)
    onesr = np.ones((1, 128), np.float32)
    return cosT, sinT, mask, rmat, ident, onesc, onesr


def _make_in_maps(inputs):
    x = np.ascontiguousarray(np.asarray(inputs["x"]), dtype=np.float32)
    Wq = np.asarray(inputs["Wq"], dtype=np.float32)
    Wk = np.asarray(inputs["Wk"], dtype=np.float32)
    Wv = np.asarray(inputs["Wv"], dtype=np.float32)
    Wo = np.asarray(inputs["Wo"], dtype=np.float32)

    xT = _round_f32r(x.reshape(BS, D).T)                    # [D, B*S]
    cosT, sinT, mask, rmat, ident, onesc, onesr = _host_consts()

    in_maps = []
    for cid in range(N_CORES):
        f0 = cid * FPC
        in_maps.append(dict(
            xT=xT,
            wq=_round_f32r(Wq[f0:f0 + FPC, :].T),
            wk=_round_f32r(Wk[f0:f0 + FPC, :].T),
            wv=_round_f32r(Wv[f0:f0 + FPC, :].T),
            wo=_round_f32r(Wo[:, f0:f0 + FPC].T),
            cos=cosT, sin=sinT, mask=mask, rmat=rmat, ident=ident,
            onesc=onesc, onesr=onesr,
        ))
    return in_maps


def kernel(x, Wq, Wk, Wv, Wo):
    in_maps = _make_in_maps(dict(x=x, Wq=Wq, Wk=Wk, Wv=Wv, Wo=Wo))
    nc = _get_nc()
    res = run_bass_kernel_spmd(nc, in_maps, core_ids=list(range(N_CORES)))
    outT = res.results[0]["outP"]
    for cid in range(1, N_CORES):
        outT = outT + res.results[cid]["outP"]
    return np.ascontiguousarray(outT.T).reshape(B, S, D)



# revision 2
# speedup vs baseline: 1.2609x; 1.2609x over previous
"""Causal self-attention with RoPE for TRN2, sharded over 8 NeuronCores.

Sharding: grid of (batch=2) x (head-groups=4). Core c handles batch c//4
and heads 4*(c%4) .. 4*(c%4)+3 (512 of 2048 features). Each core computes
q/k/v projections for its 4 heads, causal attention, and a partial output
projection through its 512 rows of Wo^T; the host sums the 4 partials per
batch. All DRAM I/O is fp16 (halves DMA + host<->device bytes); PSUM
accumulation is fp32.

Key structural choices:
  - V is projected directly into [s, d] layout (stationary = x s-block,
    moving = Wv columns), so the attention AV matmul needs no transposes.
  - RoPE head-dims are host-permuted so pair (i, i+64) becomes (2i, 2i+1);
    rotate-half is then an adjacent-pair stream_shuffle on the DVE
    (quadrant-local), with the sign folded into a pre-negated sinA.
    RoPE touches neither PE nor PSUM.
  - Softmax runs unnormalized as exp(s*scale - 4) (the bias cancels in the
    normalization); per-q sums accumulate on the DVE in fp16; the
    normalization (ones-column matmul partition reduce, reciprocal,
    ones-row broadcast matmul) is deferred so attention and projections
    can share the 8 PSUM banks.
  - Projection pass 1 (heads 2,3) is interleaved at matmul granularity
    with attention of heads 0,1; attention of heads 2,3 is interleaved
    with the output projection one s-chunk behind.
"""
import sys

sys.path.insert(0, "/opt/trn_rl_repo")

from collections import deque

import numpy as np

import concourse.bass as bass
import concourse.bacc as bacc
import concourse.mybir as mybir
import concourse.tile as tile
from concourse.bass_utils import run_bass_kernel_spmd

F32 = mybir.dt.float32
F16 = mybir.dt.float16

B, S, D, H, HD = 2, 2048, 2048, 16, 128
N_CORES = 8
HPC = 4                      # heads per core
FPC = HPC * HD               # features per core = 512
SCALE = 1.0 / float(np.sqrt(HD))
EXP_BIAS = -4.0              # exp(s*SCALE - 4): cancels in normalization
NG = 4                       # contraction groups (each 4 x 128 = 512 of K)
NSC = S // 512               # 4 s-chunks
NJT = S // 128               # 16 key tiles
SWAP_MASK = [i ^ 1 for i in range(32)]  # adjacent-pair swap, per quadrant


def build_nc():
    nc = bacc.Bacc(None, target_bir_lowering=False, debug=False)
    Exp = mybir.ActivationFunctionType.Exp

    xT_d = nc.dram_tensor("xT", [D, S], F16, kind="ExternalInput")
    wq_d = nc.dram_tensor("wq", [D, FPC], F16, kind="ExternalInput")
    wk_d = nc.dram_tensor("wk", [D, FPC], F16, kind="ExternalInput")
    wv_d = nc.dram_tensor("wv", [D, FPC], F16, kind="ExternalInput")
    wo_d = nc.dram_tensor("wo", [FPC, D], F16, kind="ExternalInput")
    cos_d = nc.dram_tensor("cos", [128, S], F16, kind="ExternalInput")
    sinA_d = nc.dram_tensor("sinA", [128, S], F16, kind="ExternalInput")
    mask_d = nc.dram_tensor("mask", [128, 896], F16, kind="ExternalInput")
    onesc_d = nc.dram_tensor("onesc", [128, 1], F16, kind="ExternalInput")
    onesr_d = nc.dram_tensor("onesr", [1, 128], F16, kind="ExternalInput")
    out_d = nc.dram_tensor("outP", [D, S], F16, kind="ExternalOutput")

    xT_r = xT_d[:].rearrange("(g t p) s -> g p t s", p=128, t=4)
    wq_r = wq_d[:].rearrange("(g t p) f -> g p t f", p=128, t=4)
    wk_r = wk_d[:].rearrange("(g t p) f -> g p t f", p=128, t=4)
    wv_r = wv_d[:].rearrange("(g t p) f -> g p t f", p=128, t=4)
    wo_r = wo_d[:].rearrange("(hb p) d -> p hb d", p=128)
    out_r = out_d[:].rearrange("(dt p) s -> dt p s", p=128)

    with tile.TileContext(nc) as tc:
        with (
            nc.allow_low_precision(reason="fp16 compute is intended"),
            tc.tile_pool(name="const", bufs=1) as constp,
            tc.tile_pool(name="xt", bufs=6) as xtp,
            tc.tile_pool(name="qkv", bufs=1) as qkvp,
            tc.tile_pool(name="attn", bufs=6) as attnp,
            tc.tile_pool(name="asum", bufs=1) as asump,
            tc.tile_pool(name="rope", bufs=2) as ropep,
            tc.tile_pool(name="small", bufs=4) as smallp,
            tc.tile_pool(name="osb", bufs=1) as osbp,
            tc.tile_pool(name="outev", bufs=4) as outevp,
            tc.tile_pool(name="pq", bufs=4, space="PSUM") as pqp,
            tc.tile_pool(name="pv", bufs=2, space="PSUM") as pvp,
            tc.tile_pool(name="psc", bufs=1, space="PSUM") as pscp,
            tc.tile_pool(name="pav", bufs=1, space="PSUM") as pavp,
        ):
            # ---- constants ----
            wq_g, wk_g, wv_g = [], [], []
            for g in range(NG):
                wqt = constp.tile([128, 4, FPC], F16, name=f"wq_g{g}")
                wkt = constp.tile([128, 4, FPC], F16, name=f"wk_g{g}")
                wvt = constp.tile([128, 4, FPC], F16, name=f"wv_g{g}")
                nc.scalar.dma_start(wqt[:], wq_r[g])
                nc.sync.dma_start(wkt[:], wk_r[g])
                nc.gpsimd.dma_start(wvt[:], wv_r[g])
                wq_g.append(wqt); wk_g.append(wkt); wv_g.append(wvt)
            cos_sb = constp.tile([128, S], F16)
            sinA_sb = constp.tile([128, S], F16)
            nc.scalar.dma_start(cos_sb[:], cos_d[:])
            nc.sync.dma_start(sinA_sb[:], sinA_d[:])
            mask_sb = constp.tile([128, 896], F16)
            onesc_sb = constp.tile([128, 1], F16)
            onesr_sb = constp.tile([1, 128], F16)
            nc.gpsimd.dma_start(mask_sb[:], mask_d[:])
            nc.scalar.dma_start(onesc_sb[:], onesc_d[:])
            nc.scalar.dma_start(onesr_sb[:], onesr_d[:])
            wo_sb = constp.tile([128, HPC, D], F16)
            expbias_sb = constp.tile([128, 1], F32, name="expbias")
            nc.gpsimd.memset(expbias_sb[:], EXP_BIAS)
            for hb in range(HPC):
                nc.gpsimd.dma_start(wo_sb[:, hb, :], wo_r[:, hb, :])

            # persistent activations
            qT = [qkvp.tile([128, S], F16, name=f"qT{h}") for h in range(HPC)]
            kT = [qkvp.tile([128, S], F16, name=f"kT{h}") for h in range(HPC)]
            # v in [s, d] layout per pass: [128 s, 16 jt, 2*128 d]
            v_h = [qkvp.tile([128, NJT, 256], F16, name=f"v_h{p}")
                   for p in range(2)]
            o_sb = osbp.tile([128, HPC, S], F16)
            at_sum = [[asump.tile([128, 512], F16, name=f"as{h}_{qc}")
                       for qc in range(NSC)] for h in range(HPC)]

            rope_q = deque()  # deferred single-op DVE thunks (pass 1 rope)

            def rope_ops(h, ss):
                """Four DVE thunks: t = t*cos + swap(t)*sinA, in place."""
                for t_ in (qT[h], kT[h]):
                    sh = ropep.tile([128, 512], F16, name="ropesh", tag="ropetmp")
                    t1 = ropep.tile([128, 512], F16, name="ropet1", tag="ropetmp2")
                    yield lambda t_=t_, sh=sh: nc.vector.stream_shuffle(
                        sh[:], t_[:, ss], SWAP_MASK)
                    yield lambda sh=sh: nc.vector.tensor_mul(
                        sh[:], sh[:], sinA_sb[:, ss])
                    yield lambda t_=t_, t1=t1: nc.vector.tensor_mul(
                        t1[:], t_[:, ss], cos_sb[:, ss])
                    yield lambda t_=t_, t1=t1, sh=sh: nc.vector.tensor_add(
                        t_[:, ss], t1[:], sh[:])

            def proj_pass(p, defer_rope):
                """Generator: projection for heads (2p, 2p+1); yields after
                each PE matmul group for interleaving."""
                heads = (2 * p, 2 * p + 1)
                vfs = slice(256 * p, 256 * p + 256)
                for sc in range(NSC):
                    ss = slice(512 * sc, 512 * sc + 512)
                    xts = []
                    for g in range(NG):
                        xt = xtp.tile([128, 4, 512], F16, name="xt", tag="xt")
                        nc.sync.dma_start(xt[:], xT_r[g, :, :, ss])
                        xts.append(xt)
                    accq = [pqp.tile([128, 512], F32, name=f"accq{j}", tag="pacq")
                            for j in range(2)]
                    acck = [pqp.tile([128, 512], F32, name=f"acck{j}", tag="pacq")
                            for j in range(2)]
                    # q/k: stationary = weight slice, moving = x chunk
                    for g in range(NG):
                        for t in range(4):
                            st = (g, t) == (0, 0)
                            sp = (g, t) == (NG - 1, 3)
                            for j, h in enumerate(heads):
                                fs = slice(128 * h, 128 * h + 128)
                                nc.tensor.matmul(accq[j][:], wq_g[g][:, t, fs],
                                                 xts[g][:, t, :],
                                                 start=st, stop=sp)
                                nc.tensor.matmul(acck[j][:], wk_g[g][:, t, fs],
                                                 xts[g][:, t, :],
                                                 start=st, stop=sp)
                            yield
                    for j, h in enumerate(heads):
                        nc.scalar.copy(qT[h][:, ss], accq[j][:])
                        nc.scalar.copy(kT[h][:, ss], acck[j][:])
                    # v: stationary = x s-block, moving = both heads' columns
                    for sb in range(4):
                        jt = 4 * sc + sb
                        sbs = slice(128 * sb, 128 * sb + 128)
                        accv = pvp.tile([128, 512], F32, name="accv", tag="pacv")
                        for g in range(NG):
                            for t in range(4):
                                st = (g, t) == (0, 0)
                                sp = (g, t) == (NG - 1, 3)
                                nc.tensor.matmul(accv[:, 0:256],
                                                 xts[g][:, t, sbs],
                                                 wv_g[g][:, t, vfs],
                                                 start=st, stop=sp)
                            yield
                        nc.scalar.copy(v_h[p][:, jt, :], accv[:, 0:256])
                    # RoPE on this chunk's q/k (DVE only)
                    for h in heads:
                        ops = rope_ops(h, ss)
                        if defer_rope:
                            rope_q.extend(ops)
                        else:
                            for op in ops:
                                op()
                    yield

            def attn_steps(h, qc):
                """Generator: attention for (head, q-chunk); yields per
                key-tile step."""
                p, hh = divmod(h, 2)
                qs = slice(512 * qc, 512 * qc + 512)
                ps_av = pavp.tile([128, 512], F32, name="ps_av", tag="pav")
                njt = 4 * qc + 4
                for jt in range(njt):
                    js = slice(128 * jt, 128 * jt + 128)
                    ps_sc = pscp.tile([128, 512], F32, name="ps_sc", tag="psc")
                    nc.tensor.matmul(ps_sc[:], kT[h][:, js], qT[h][:, qs],
                                     start=True, stop=True)
                    if jt == 0:
                        at = at_sum[h][qc]
                    else:
                        at = attnp.tile([128, 512], F16, name="at", tag="at")
                    nc.scalar.activation(at[:], ps_sc[:], Exp,
                                         bias=expbias_sb[:], scale=SCALE)
                    if jt >= 4 * qc:
                        mi = 384 - 128 * (jt - 4 * qc)
                        nc.vector.tensor_mul(at[:], at[:],
                                             mask_sb[:, mi:mi + 512])
                    if jt > 0:
                        nc.vector.tensor_add(at_sum[h][qc][:],
                                             at_sum[h][qc][:], at[:])
                    vs = slice(128 * hh, 128 * hh + 128)
                    nc.tensor.matmul(ps_av[:], v_h[p][:, jt, vs], at[:],
                                     start=jt == 0, stop=jt == njt - 1)
                    yield
                # unnormalized eviction; normalization is deferred
                nc.scalar.copy(o_sb[:, h, qs], ps_av[:])

            def normalize(h, qc):
                qs = slice(512 * qc, 512 * qc + 512)
                ps_sum = pvp.tile([1, 512], F32, name="ps_sum", tag="pacv")
                nc.tensor.matmul(ps_sum[:], onesc_sb[:], at_sum[h][qc][:],
                                 start=True, stop=True)
                recip = smallp.tile([1, 512], F16, name="recip", tag="recip")
                nc.vector.reciprocal(recip[:], ps_sum[:])
                ps_bc = pvp.tile([128, 512], F32, name="ps_bc", tag="pacv")
                nc.tensor.matmul(ps_bc[:], onesr_sb[:], recip[:],
                                 start=True, stop=True)
                recipT = smallp.tile([128, 512], F16, name="recipT", tag="recipT")
                nc.scalar.copy(recipT[:], ps_bc[:])
                nc.vector.tensor_mul(o_sb[:, h, qs], o_sb[:, h, qs],
                                     recipT[:])

            def outproj_units(sc):
                """Generator: output projection for s-chunk sc; yields per
                128-row output block."""
                ss = slice(512 * sc, 512 * sc + 512)
                for dt in range(D // 128):
                    ds = slice(128 * dt, 128 * dt + 128)
                    ps_o = pqp.tile([128, 512], F32, name="ps_o", tag="pacq")
                    for hb in range(HPC):
                        nc.tensor.matmul(ps_o[:], wo_sb[:, hb, ds],
                                         o_sb[:, hb, ss],
                                         start=hb == 0, stop=hb == HPC - 1)
                    outt = outevp.tile([128, 512], F16, name="outt", tag="outt")
                    if dt % 2 == 0:
                        nc.vector.tensor_copy(outt[:], ps_o[:])
                    else:
                        nc.scalar.copy(outt[:], ps_o[:])
                    nc.gpsimd.dma_start(out_r[dt, :, ss], outt[:])
                    yield

            def interleave(gen_a, gen_b, ratio=1):
                """Emit from two generators alternately (ratio a-steps per
                b-step); drains the deferred-rope queue one op per step."""
                a_live = b_live = True
                while a_live or b_live:
                    if a_live:
                        for _ in range(ratio):
                            try:
                                next(gen_a)
                            except StopIteration:
                                a_live = False
                                break
                    if rope_q:
                        rope_q.popleft()()
                    if b_live:
                        try:
                            next(gen_b)
                        except StopIteration:
                            b_live = False

            # ---- phase 1: projection pass 0 (heads 0, 1) ----
            for _ in proj_pass(0, defer_rope=False):
                pass

            # ---- phase 2: projection pass 1 interleaved with attention
            #      of heads 0, 1 ----
            att01 = (s for h in (0, 1) for qc in range(NSC)
                     for s in attn_steps(h, qc))
            interleave(proj_pass(1, defer_rope=True), att01, ratio=2)
            while rope_q:
                rope_q.popleft()()

            # ---- phases 3+4: normalize heads 0,1; attention heads 2,3
            #      interleaved with the output projection (one s-chunk
            #      behind) ----
            def stream_a():
                for h in (0, 1):
                    for qc in range(NSC):
                        normalize(h, qc)
                for qc in range(NSC):
                    for h in (2, 3):
                        for s in attn_steps(h, qc):
                            yield
                    for h in (2, 3):
                        normalize(h, qc)
                    yield ("chunk_done", qc)

            def stream_b(markers):
                for sc in range(NSC):
                    while sc not in markers:
                        yield False
                    for _ in outproj_units(sc):
                        yield True

            markers = set()
            gen_a, gen_b = stream_a(), stream_b(markers)
            a_live = True
            while True:
                if a_live:
                    try:
                        r = next(gen_a)
                        if isinstance(r, tuple):
                            markers.add(r[1])
                    except StopIteration:
                        a_live = False
                try:
                    emitted = next(gen_b)
                except StopIteration:
                    break
                if not a_live and not emitted:
                    raise RuntimeError("outproj stalled")  # pragma: no cover

    nc.compile()
    return nc


_NC_CACHE = None


def _get_nc():
    global _NC_CACHE
    if _NC_CACHE is None:
        _NC_CACHE = build_nc()
    return _NC_CACHE


def _host_consts():
    hd = HD
    inv_freq = 1.0 / (10000.0 ** (np.arange(0, hd, 2, dtype=np.float32) / hd))
    t = np.arange(S, dtype=np.float32)
    freqs = np.outer(t, inv_freq)
    emb = np.concatenate([freqs, freqs], axis=-1)        # [S, hd]
    cos = np.cos(emb).T                                   # [hd, S]
    sin = np.sin(emb).T
    # head-dim permutation: new[2m] = old[m], new[2m+1] = old[m+64]
    perm = np.empty(hd, dtype=np.int64)
    perm[0::2] = np.arange(64)
    perm[1::2] = np.arange(64) + 64
    cosP = cos[perm]
    sinP = sin[perm]
    # rotate-half in permuted layout = adjacent swap, sign folded in:
    # rot[2m] = -t[2m+1], rot[2m+1] = +t[2m]
    sinA = sinP.copy()
    sinA[0::2] *= -1.0
    r = np.arange(128)[:, None]
    u = np.arange(896)[None, :]
    mask = (u >= r + 384).astype(np.float32)
    onesc = np.ones((128, 1), np.float32)
    onesr = np.ones((1, 128), np.float32)
    return (cosP.astype(np.float16), sinA.astype(np.float16),
            mask.astype(np.float16), onesc.astype(np.float16),
            onesr.astype(np.float16), perm)


def _make_in_maps(inputs):
    x = np.asarray(inputs["x"], dtype=np.float32)
    Wq = np.asarray(inputs["Wq"], dtype=np.float32)
    Wk = np.asarray(inputs["Wk"], dtype=np.float32)
    Wv = np.asarray(inputs["Wv"], dtype=np.float32)
    Wo = np.asarray(inputs["Wo"], dtype=np.float32)

    cosP, sinA, mask, onesc, onesr, perm = _host_consts()
    xT = [np.ascontiguousarray(x[b].T).astype(np.float16) for b in range(B)]

    in_maps = []
    for cid in range(N_CORES):
        b, hg = divmod(cid, HPC)
        f0 = hg * FPC
        # rows of W within this shard; RoPE head-dim permutation applied
        # per head for wq/wk (q/k stay permuted; scores are invariant)
        rq = np.concatenate([f0 + 128 * h + perm for h in range(HPC)])
        rplain = np.arange(f0, f0 + FPC)
        in_maps.append(dict(
            xT=xT[b],
            wq=np.ascontiguousarray(Wq[rq, :].T).astype(np.float16),
            wk=np.ascontiguousarray(Wk[rq, :].T).astype(np.float16),
            wv=np.ascontiguousarray(Wv[rplain, :].T).astype(np.float16),
            wo=np.ascontiguousarray(Wo[:, rplain].T).astype(np.float16),
            cos=cosP, sinA=sinA, mask=mask, onesc=onesc, onesr=onesr,
        ))
    return in_maps


def kernel(x, Wq, Wk, Wv, Wo):
    in_maps = _make_in_maps(dict(x=x, Wq=Wq, Wk=Wk, Wv=Wv, Wo=Wo))
    nc = _get_nc()
    res = run_bass_kernel_spmd(nc, in_maps, core_ids=list(range(N_CORES)))
    out = np.empty((B, S, D), dtype=np.float32)
    for b in range(B):
        acc = res.results[4 * b]["outP"].astype(np.float32)
        for hg in range(1, HPC):
            acc = acc + res.results[4 * b + hg]["outP"].astype(np.float32)
        out[b] = acc.T
    return out


# revision 3
# speedup vs baseline: 1.5334x; 1.2161x over previous
"""Causal self-attention with RoPE for TRN2, sharded over 8 NeuronCores.

Sharding: grid of (batch=2) x (head-groups=4). Core c handles batch c//4
and heads 4*(c%4) .. 4*(c%4)+3 (512 of 2048 features). Each core computes
q/k/v projections for its 4 heads, causal attention, and a partial output
projection through its 512 rows of Wo^T; the host sums the 4 partials per
batch. All DRAM I/O is fp16 (halves DMA + host<->device bytes); PSUM
accumulation is fp32.

Key structural choices:
  - V is projected directly into [s, d] layout (stationary = x s-block,
    moving = Wv columns), so the attention AV matmul needs no transposes.
  - RoPE head-dims are host-permuted so pair (i, i+64) becomes (2i, 2i+1);
    rotate-half is then an adjacent-pair stream_shuffle on the DVE
    (quadrant-local), with the sign folded into a pre-negated sinA.
    RoPE touches neither PE nor PSUM.
  - Softmax runs unnormalized as exp(s*scale - 4) (the bias cancels in the
    normalization); per-q sums accumulate on the DVE in fp16; the
    normalization (ones-column matmul partition reduce, reciprocal,
    ones-row broadcast matmul) is deferred so attention and projections
    can share the 8 PSUM banks.
  - Projection pass 1 (heads 2,3) is interleaved at matmul granularity
    with attention of heads 0,1; attention of heads 2,3 is interleaved
    with the output projection one s-chunk behind.
"""
import sys

sys.path.insert(0, "/opt/trn_rl_repo")

from collections import deque

import numpy as np

import concourse.bass as bass
import concourse.bacc as bacc
import concourse.mybir as mybir
import concourse.tile as tile
from concourse.bass_utils import run_bass_kernel_spmd

F32 = mybir.dt.float32
F16 = mybir.dt.float16

B, S, D, H, HD = 2, 2048, 2048, 16, 128
N_CORES = 8
HPC = 4                      # heads per core
FPC = HPC * HD               # features per core = 512
SCALE = 1.0 / float(np.sqrt(HD))
EXP_BIAS = -4.0              # exp(s*SCALE - 4): cancels in normalization
NG = 4                       # contraction groups (each 4 x 128 = 512 of K)
NSC = S // 512               # 4 s-chunks
NJT = S // 128               # 16 key tiles
SWAP_MASK = [i ^ 1 for i in range(32)]  # adjacent-pair swap, per quadrant


def build_nc():
    nc = bacc.Bacc(None, target_bir_lowering=False, debug=False)
    Exp = mybir.ActivationFunctionType.Exp

    xT_d = nc.dram_tensor("xT", [D, S], F16, kind="ExternalInput")
    wq_d = nc.dram_tensor("wq", [D, FPC], F16, kind="ExternalInput")
    wk_d = nc.dram_tensor("wk", [D, FPC], F16, kind="ExternalInput")
    wv_d = nc.dram_tensor("wv", [D, FPC], F16, kind="ExternalInput")
    wo_d = nc.dram_tensor("wo", [FPC, D], F16, kind="ExternalInput")
    cos_d = nc.dram_tensor("cos", [128, S], F16, kind="ExternalInput")
    sinA_d = nc.dram_tensor("sinA", [128, S], F16, kind="ExternalInput")
    mask_d = nc.dram_tensor("mask", [128, 896], F16, kind="ExternalInput")
    onesc_d = nc.dram_tensor("onesc", [128, 1], F16, kind="ExternalInput")
    onesr_d = nc.dram_tensor("onesr", [1, 128], F16, kind="ExternalInput")
    out_d = nc.dram_tensor("outP", [D, S], F16, kind="ExternalOutput")

    xT_r = xT_d[:].rearrange("(g t p) s -> g p t s", p=128, t=4)
    wq_r = wq_d[:].rearrange("(g t p) f -> g p t f", p=128, t=4)
    wk_r = wk_d[:].rearrange("(g t p) f -> g p t f", p=128, t=4)
    wv_r = wv_d[:].rearrange("(g t p) f -> g p t f", p=128, t=4)
    wo_r = wo_d[:].rearrange("(hb p) d -> p hb d", p=128)
    out_r = out_d[:].rearrange("(dt p) s -> dt p s", p=128)

    with tile.TileContext(nc) as tc:
        with (
            nc.allow_low_precision(reason="fp16 compute is intended"),
            tc.tile_pool(name="const", bufs=1) as constp,
            tc.tile_pool(name="xt", bufs=6) as xtp,
            tc.tile_pool(name="qkv", bufs=1) as qkvp,
            tc.tile_pool(name="attn", bufs=6) as attnp,
            tc.tile_pool(name="asum", bufs=1) as asump,
            tc.tile_pool(name="rope", bufs=2) as ropep,
            tc.tile_pool(name="small", bufs=4) as smallp,
            tc.tile_pool(name="osb", bufs=1) as osbp,
            tc.tile_pool(name="outev", bufs=4) as outevp,
            tc.tile_pool(name="pq", bufs=3, space="PSUM") as pqp,
            tc.tile_pool(name="pv", bufs=2, space="PSUM") as pvp,
            tc.tile_pool(name="psc", bufs=2, space="PSUM") as pscp,
            tc.tile_pool(name="pav", bufs=1, space="PSUM") as pavp,
        ):
            # ---- constants ----
            # wv loads are deferred into proj_pass(0)'s first chunk so the
            # Pool queue serves the first x tiles immediately.
            wq_g, wk_g, wv_g = [], [], []
            for g in range(NG):
                wqt = constp.tile([128, 4, FPC], F16, name=f"wq_g{g}")
                wkt = constp.tile([128, 4, FPC], F16, name=f"wk_g{g}")
                wvt = constp.tile([128, 4, FPC], F16, name=f"wv_g{g}")
                nc.scalar.dma_start(wqt[:], wq_r[g])
                nc.sync.dma_start(wkt[:], wk_r[g])
                wq_g.append(wqt); wk_g.append(wkt); wv_g.append(wvt)
            cos_sb = constp.tile([128, S], F16)
            sinA_sb = constp.tile([128, S], F16)
            nc.scalar.dma_start(cos_sb[:], cos_d[:])
            nc.sync.dma_start(sinA_sb[:], sinA_d[:])
            mask_sb = constp.tile([128, 896], F16)
            onesc_sb = constp.tile([128, 1], F16)
            onesr_sb = constp.tile([1, 128], F16)
            nc.sync.dma_start(mask_sb[:], mask_d[:])
            nc.scalar.dma_start(onesc_sb[:], onesc_d[:])
            nc.scalar.dma_start(onesr_sb[:], onesr_d[:])
            wo_sb = constp.tile([128, HPC, D], F16)
            expbias_sb = constp.tile([128, 1], F32, name="expbias")
            nc.gpsimd.memset(expbias_sb[:], EXP_BIAS)
            for hb in range(HPC):
                nc.sync.dma_start(wo_sb[:, hb, :], wo_r[:, hb, :])

            # persistent activations
            qT = [qkvp.tile([128, S], F16, name=f"qT{h}") for h in range(HPC)]
            kT = [qkvp.tile([128, S], F16, name=f"kT{h}") for h in range(HPC)]
            # v in [s, d] layout per pass: [128 s, 16 jt, 2*128 d]
            v_h = [qkvp.tile([128, NJT, 256], F16, name=f"v_h{p}")
                   for p in range(2)]
            o_sb = osbp.tile([128, HPC, S], F16)
            at_sum = [[asump.tile([128, 512], F16, name=f"as{h}_{qc}")
                       for qc in range(NSC)] for h in range(HPC)]

            rope_q = deque()  # deferred single-op DVE thunks (pass 1 rope)

            def rope_ops(h, ss):
                """Four DVE thunks: t = t*cos + swap(t)*sinA, in place."""
                for t_ in (qT[h], kT[h]):
                    sh = ropep.tile([128, 512], F16, name="ropesh", tag="ropetmp")
                    t1 = ropep.tile([128, 512], F16, name="ropet1", tag="ropetmp2")
                    yield lambda t_=t_, sh=sh: nc.vector.stream_shuffle(
                        sh[:], t_[:, ss], SWAP_MASK)
                    yield lambda sh=sh: nc.vector.tensor_mul(
                        sh[:], sh[:], sinA_sb[:, ss])
                    yield lambda t_=t_, t1=t1: nc.vector.tensor_mul(
                        t1[:], t_[:, ss], cos_sb[:, ss])
                    yield lambda t_=t_, t1=t1, sh=sh: nc.vector.tensor_add(
                        t_[:, ss], t1[:], sh[:])

            def proj_pass(p, defer_rope):
                """Generator: projection for heads (2p, 2p+1); yields after
                each PE matmul group for interleaving."""
                heads = (2 * p, 2 * p + 1)
                vfs = slice(256 * p, 256 * p + 256)
                for sc in range(NSC):
                    ss = slice(512 * sc, 512 * sc + 512)
                    xts = []
                    xeng = nc.gpsimd if (p == 0 and sc == 0) else nc.sync
                    for g in range(NG):
                        xt = xtp.tile([128, 4, 512], F16, name="xt", tag="xt")
                        xeng.dma_start(xt[:], xT_r[g, :, :, ss])
                        xts.append(xt)
                    if p == 0 and sc == 0:
                        for g in range(NG):
                            nc.gpsimd.dma_start(wv_g[g][:], wv_r[g])
                    # q then k: two sweeps of two accumulators each (the
                    # pacq ring is only 3 deep; the score ring needs 2)
                    for wg, dst in ((wq_g, qT), (wk_g, kT)):
                        acc = [pqp.tile([128, 512], F32, name=f"acc{j}",
                                        tag="pacq") for j in range(2)]
                        for g in range(NG):
                            for t in range(4):
                                st = (g, t) == (0, 0)
                                sp = (g, t) == (NG - 1, 3)
                                for j, h in enumerate(heads):
                                    fs = slice(128 * h, 128 * h + 128)
                                    nc.tensor.matmul(acc[j][:],
                                                     wg[g][:, t, fs],
                                                     xts[g][:, t, :],
                                                     start=st, stop=sp)
                                yield
                        for j, h in enumerate(heads):
                            nc.scalar.copy(dst[h][:, ss], acc[j][:])
                    # v: stationary = x s-block, moving = both heads' columns
                    for sb in range(4):
                        jt = 4 * sc + sb
                        sbs = slice(128 * sb, 128 * sb + 128)
                        accv = pvp.tile([128, 512], F32, name="accv", tag="pacv")
                        for g in range(NG):
                            for t in range(4):
                                st = (g, t) == (0, 0)
                                sp = (g, t) == (NG - 1, 3)
                                nc.tensor.matmul(accv[:, 0:256],
                                                 xts[g][:, t, sbs],
                                                 wv_g[g][:, t, vfs],
                                                 start=st, stop=sp)
                            yield
                        nc.scalar.copy(v_h[p][:, jt, :], accv[:, 0:256])
                    # RoPE on this chunk's q/k (DVE only)
                    for h in heads:
                        ops = rope_ops(h, ss)
                        if defer_rope:
                            rope_q.extend(ops)
                        else:
                            for op in ops:
                                op()
                    yield

            def attn_steps(h, qc, avpool=None, avtag="pav"):
                """Generator: attention for (head, q-chunk); yields per
                key-tile step. Below the diagonal block-row the key tiles
                span all 512 q columns; the 4x4 diagonal block runs per
                128-wide q sub-column so above-diagonal work is skipped."""
                p, hh = divmod(h, 2)
                vs = slice(128 * hh, 128 * hh + 128)
                qs = slice(512 * qc, 512 * qc + 512)
                ps_av = (avpool or pavp).tile([128, 512], F32,
                                              name="ps_av", tag=avtag)
                asum = at_sum[h][qc]
                for jt in range(4 * qc):
                    js = slice(128 * jt, 128 * jt + 128)
                    ps_sc = pscp.tile([128, 512], F32, name="ps_sc", tag="psc")
                    nc.tensor.matmul(ps_sc[:], kT[h][:, js], qT[h][:, qs],
                                     start=True, stop=True)
                    if jt == 0:
                        at = asum[:]
                    else:
                        att = attnp.tile([128, 512], F16, name="at", tag="at")
                        at = att[:]
                    nc.scalar.activation(at, ps_sc[:], Exp,
                                         bias=expbias_sb[:], scale=SCALE)
                    if jt > 0:
                        nc.vector.tensor_add(asum[:], asum[:], at)
                    nc.tensor.matmul(ps_av[:], v_h[p][:, jt, vs], at,
                                     start=jt == 0, stop=False,
                                     skip_group_check=True)
                    yield
                for qi in range(4):
                    q1 = slice(512 * qc + 128 * qi, 512 * qc + 128 * qi + 128)
                    c1 = slice(128 * qi, 128 * qi + 128)
                    for dj in range(qi + 1):
                        jt = 4 * qc + dj
                        js = slice(128 * jt, 128 * jt + 128)
                        ps_sc = pscp.tile([128, 512], F32, name="ps_sc",
                                          tag="psc")
                        nc.tensor.matmul(ps_sc[:, 0:128], kT[h][:, js],
                                         qT[h][:, q1], start=True, stop=True)
                        first = qc == 0 and dj == 0
                        if first:
                            at = asum[:, c1]
                        else:
                            att = attnp.tile([128, 512], F16, name="at",
                                             tag="at")
                            at = att[:, 0:128]
                        nc.scalar.activation(at, ps_sc[:, 0:128], Exp,
                                             bias=expbias_sb[:], scale=SCALE)
                        if dj == qi:
                            nc.vector.tensor_mul(at, at, mask_sb[:, 384:512])
                        if not first:
                            nc.vector.tensor_add(asum[:, c1], asum[:, c1], at)
                        nc.tensor.matmul(ps_av[:, c1], v_h[p][:, jt, vs], at,
                                         start=first, stop=dj == qi,
                                         skip_group_check=True)
                        yield
                # unnormalized eviction; normalization is deferred
                nc.scalar.copy(o_sb[:, h, qs], ps_av[:])

            def normalize(h, qc):
                qs = slice(512 * qc, 512 * qc + 512)
                ps_sum = pqp.tile([1, 512], F32, name="ps_sum", tag="pacq")
                nc.tensor.matmul(ps_sum[:], onesc_sb[:], at_sum[h][qc][:],
                                 start=True, stop=True)
                recip = smallp.tile([1, 512], F16, name="recip", tag="recip")
                nc.vector.reciprocal(recip[:], ps_sum[:])
                ps_bc = pqp.tile([128, 512], F32, name="ps_bc", tag="pacq")
                nc.tensor.matmul(ps_bc[:], onesr_sb[:], recip[:],
                                 start=True, stop=True)
                recipT = smallp.tile([128, 512], F16, name="recipT", tag="recipT")
                nc.scalar.copy(recipT[:], ps_bc[:])
                nc.vector.tensor_mul(o_sb[:, h, qs], o_sb[:, h, qs],
                                     recipT[:])

            def outproj_units(sc):
                """Generator: output projection for s-chunk sc; yields per
                128-row output block."""
                ss = slice(512 * sc, 512 * sc + 512)
                for dt in range(D // 128):
                    ds = slice(128 * dt, 128 * dt + 128)
                    ps_o = pqp.tile([128, 512], F32, name="ps_o", tag="pacq")
                    for hb in range(HPC):
                        nc.tensor.matmul(ps_o[:], wo_sb[:, hb, ds],
                                         o_sb[:, hb, ss],
                                         start=hb == 0, stop=hb == HPC - 1)
                    outt = outevp.tile([128, 512], F16, name="outt", tag="outt")
                    if dt % 2 == 0:
                        nc.vector.tensor_copy(outt[:], ps_o[:])
                    else:
                        nc.scalar.copy(outt[:], ps_o[:])
                    nc.gpsimd.dma_start(out_r[dt, :, ss], outt[:])
                    yield

            # ---- phase 1: projection pass 0 (heads 0, 1) ----
            for _ in proj_pass(0, defer_rope=False):
                pass

            # ---- phase 2: projection pass 1 interleaved with attention
            #      of heads 0, 1, credit-paced (132 proj yields per 80
            #      attention steps); deferred pass-1 rope ops drain one
            #      per step ----
            att01 = (s for h in (0, 1) for qc in range(NSC)
                     for s in attn_steps(h, qc))
            proj1 = proj_pass(1, defer_rope=True)
            a_live = b_live = True
            credit = 0.0
            while a_live or b_live:
                if a_live:
                    credit += 196.0 / 128.0
                    while credit >= 1.0 and a_live:
                        try:
                            next(proj1)
                            credit -= 1.0
                        except StopIteration:
                            a_live = False
                if rope_q:
                    rope_q.popleft()()
                if b_live:
                    try:
                        next(att01)
                    except StopIteration:
                        b_live = False
            while rope_q:
                rope_q.popleft()()

            # ---- phases 3+4: attention heads 2,3; the qc0 window is
            #      filled with the deferred normalizations of heads 0,1;
            #      later windows interleave the output projection one
            #      s-chunk behind, paced at units/steps ----
            # Heads 2 and 3 run as two interleaved chains so one head's
            # matmuls hide the other head's exp latency; head 3's ps_av
            # borrows a bank from the (otherwise idle) pq pool.
            fill = deque((h, qc) for h in (0, 1) for qc in range(NSC))
            outgen = None
            for qc in range(NSC):
                credit = 0.0
                units_left = 16
                pairs_left = 4 * qc + 10
                g2 = attn_steps(2, qc)
                g3 = attn_steps(3, qc, avpool=pvp, avtag="pacv")
                for s2, s3 in zip(g2, g3):
                    for _ in range(2):
                        if fill:
                            nh, nqc = fill.popleft()
                            normalize(nh, nqc)
                        elif outgen is not None:
                            credit += 0.5 * units_left / max(pairs_left, 1)
                            while credit >= 1.0 and outgen is not None:
                                try:
                                    next(outgen)
                                    units_left -= 1
                                    credit -= 1.0
                                except StopIteration:
                                    outgen = None
                    pairs_left -= 1
                for g in (g2, g3):
                    for _ in g:
                        pass
                if outgen is not None:
                    for _ in outgen:
                        pass
                normalize(2, qc)
                normalize(3, qc)
                outgen = outproj_units(qc)
            for _ in outgen:
                pass

    nc.compile()
    return nc


_NC_CACHE = None


def _get_nc():
    global _NC_CACHE
    if _NC_CACHE is None:
        _NC_CACHE = build_nc()
    return _NC_CACHE


def _host_consts():
    hd = HD
    inv_freq = 1.0 / (10000.0 ** (np.arange(0, hd, 2, dtype=np.float32) / hd))
    t = np.arange(S, dtype=np.float32)
    freqs = np.outer(t, inv_freq)
    emb = np.concatenate([freqs, freqs], axis=-1)        # [S, hd]
    cos = np.cos(emb).T                                   # [hd, S]
    sin = np.sin(emb).T
    # head-dim permutation: new[2m] = old[m], new[2m+1] = old[m+64]
    perm = np.empty(hd, dtype=np.int64)
    perm[0::2] = np.arange(64)
    perm[1::2] = np.arange(64) + 64
    cosP = cos[perm]
    sinP = sin[perm]
    # rotate-half in permuted layout = adjacent swap, sign folded in:
    # rot[2m] = -t[2m+1], rot[2m+1] = +t[2m]
    sinA = sinP.copy()
    sinA[0::2] *= -1.0
    r = np.arange(128)[:, None]
    u = np.arange(896)[None, :]
    mask = (u >= r + 384).astype(np.float32)
    onesc = np.ones((128, 1), np.float32)
    onesr = np.ones((1, 128), np.float32)
    return (cosP.astype(np.float16), sinA.astype(np.float16),
            mask.astype(np.float16), onesc.astype(np.float16),
            onesr.astype(np.float16), perm)


def _make_in_maps(inputs):
    x = np.asarray(inputs["x"], dtype=np.float32)
    Wq = np.asarray(inputs["Wq"], dtype=np.float32)
    Wk = np.asarray(inputs["Wk"], dtype=np.float32)
    Wv = np.asarray(inputs["Wv"], dtype=np.float32)
    Wo = np.asarray(inputs["Wo"], dtype=np.float32)

    cosP, sinA, mask, onesc, onesr, perm = _host_consts()
    xT = [np.ascontiguousarray(x[b].T).astype(np.float16) for b in range(B)]

    in_maps = []
    for cid in range(N_CORES):
        b, hg = divmod(cid, HPC)
        f0 = hg * FPC
        # rows of W within this shard; RoPE head-dim permutation applied
        # per head for wq/wk (q/k stay permuted; scores are invariant)
        rq = np.concatenate([f0 + 128 * h + perm for h in range(HPC)])
        rplain = np.arange(f0, f0 + FPC)
        in_maps.append(dict(
            xT=xT[b],
            wq=np.ascontiguousarray(Wq[rq, :].T).astype(np.float16),
            wk=np.ascontiguousarray(Wk[rq, :].T).astype(np.float16),
            wv=np.ascontiguousarray(Wv[rplain, :].T).astype(np.float16),
            wo=np.ascontiguousarray(Wo[:, rplain].T).astype(np.float16),
            cos=cosP, sinA=sinA, mask=mask, onesc=onesc, onesr=onesr,
        ))
    return in_maps


def kernel(x, Wq, Wk, Wv, Wo):
    in_maps = _make_in_maps(dict(x=x, Wq=Wq, Wk=Wk, Wv=Wv, Wo=Wo))
    nc = _get_nc()
    res = run_bass_kernel_spmd(nc, in_maps, core_ids=list(range(N_CORES)))
    out = np.empty((B, S, D), dtype=np.float32)
    for b in range(B):
        acc = res.results[4 * b]["outP"].astype(np.float32)
        for hg in range(1, HPC):
            acc = acc + res.results[4 * b + hg]["outP"].astype(np.float32)
        out[b] = acc.T
    return out
